# revision 12
# baseline (speedup 1.0000x reference)
"""Trainium2 Bass kernel for nn_CGCA_branch (gnn_message_passing).

Math: every op between x and the relu is linear and commutes with the global
average pool, so conv1 / grouped-conv2 / fc1 / (1/S mean) *and the adjacency
softmax matmul* all fold on the host into a single [J, C] matrix:

    gc[j, n] = Wg @ sum_s(x[n, :, s]),   Wg = softmax(adj) @ fc1 @ M2 @ (w1/S)

The device kernel is then a 51 MB/core spatial-sum stream (HBM/DMA-bound)
plus tiny per-chunk matmuls, a relu, one [8,512] matmul (split in halves to
pipeline with the sigmoid + store) and a sigmoid.

Streaming: one 1.6 MB DMA per (sample, 128-channel chunk) on the qSync HWDGE
ring, in order; chunk reduces alternate between DVE (reduce_sum) and ACT
(Copy with accum_out) so reduce throughput is ~2.2x one engine and every
engine's in-order queue matches data-arrival order (out-of-order engine
queues were measured to stall the DMA ring on slot-reuse waits).  The last
sample's final chunks are split into small pieces so the post-stream tail is
short.  ACT uses only Copy and Sigmoid - one table set, loaded once at start.

Sharding: pure data parallel - batch 64 split into 8 shards of 8 samples,
one per NeuronCore; weights replicated.
"""

import numpy as np

import concourse.bass as bass
import concourse.bacc as bacc
from concourse import mybir
from concourse.bass_utils import run_bass_kernel_spmd
from concourse.tile import TileContext
from contextlib import ExitStack

# ---- problem constants (hardcoded per harness contract) ----
N, C, H, W = 64, 512, 56, 56
S = H * W                      # 3136 spatial positions
J, CA, G = 17, 272, 16
NCORES = 8
NL = N // NCORES               # 8 samples per core
CT = C // 128                  # 4 channel chunks of 128
NEG = -9e15

_ADJ = np.array([
    [1,1,0,0,0,0,0,0,0,0,0,0,0,0,0,0,0],[1,1,1,0,0,0,0,0,0,0,0,0,0,0,0,0,0],
    [0,1,1,0,0,0,1,0,0,0,0,0,0,0,0,0,0],[0,0,0,1,1,0,1,0,0,0,0,0,0,0,0,0,0],
    [0,0,0,1,1,1,0,0,0,0,0,0,0,0,0,0,0],[0,0,0,0,1,1,0,0,0,0,0,0,0,0,0,0,0],
    [0,0,1,1,0,0,1,1,0,0,0,0,0,0,0,0,0],[0,0,0,0,0,0,1,1,1,0,0,0,0,0,0,0,0],
    [0,0,0,0,0,0,0,1,1,0,0,1,1,0,0,0,1],[0,0,0,0,0,0,0,0,0,1,0,0,0,0,0,0,1],
    [0,0,0,0,0,0,0,0,0,0,1,1,0,0,0,0,0],[0,0,0,0,0,0,0,0,0,0,1,1,1,0,0,0,0],
    [0,0,0,0,0,0,0,0,1,0,0,1,1,0,0,0,0],[0,0,0,0,0,0,0,0,1,0,0,0,0,1,1,0,0],
    [0,0,0,0,0,0,0,0,0,0,0,0,0,1,1,1,0],[0,0,0,0,0,0,0,0,0,0,0,0,0,0,1,1,0],
    [0,0,0,0,0,0,0,0,1,1,0,0,0,0,0,0,1]], dtype=np.int32)
NZ_IDX = np.flatnonzero(_ADJ)  # 49 entries

F32 = mybir.dt.float32
ACT_COPY = mybir.ActivationFunctionType.Copy
ACT_SIGMOID = mybir.ActivationFunctionType.Sigmoid
_NC_CACHE = {}


def _build_nc() -> bass.Bass:
    nc = bacc.Bacc(None, enable_partition_id=False)
    x_d = nc.declare_dram_parameter("x", [NL, C, S], F32, isOutput=False)
    wgt_d = nc.declare_dram_parameter("wgt", [128, CT, J], F32, isOutput=False)
    fc2t_d = nc.declare_dram_parameter("fc2t", [J, C], F32, isOutput=False)
    out_d = nc.declare_dram_parameter("out", [NL, C], F32, isOutput=True)

    with TileContext(nc) as tc, ExitStack() as ctx:
        xpool = ctx.enter_context(tc.tile_pool(name="xpool", bufs=6))
        singles = ctx.enter_context(tc.tile_pool(name="singles", bufs=1))
        smalls = ctx.enter_context(tc.tile_pool(name="smalls", bufs=3))
        psum = ctx.enter_context(tc.tile_pool(name="psum", bufs=2, space="PSUM"))

        # replicated weights on the SWDGE queue so the HWDGE ring carries
        # only the x stream
        wgt_sb = singles.tile([128, CT, J], F32)
        nc.gpsimd.dma_start(out=wgt_sb, in_=wgt_d[:, :, :])
        fc2t_sb = singles.tile([J, C], F32)
        nc.gpsimd.dma_start(out=fc2t_sb, in_=fc2t_d[:, :])

        # ---- stream x, spatial-sum per (sample, channel-chunk) ----
        xm_sb = singles.tile([128, CT, NL], F32)        # xm[p, ct, n]
        stage = singles.tile([128, 12], F32)            # split-piece partials
        scratch = singles.tile([128, S], F32)           # dummy out for ACT accum
        gc_ps = psum.tile([J, NL], F32, tag="gc")       # gc accumulator
        xv = x_d[:, :, :].rearrange("n (ct p) s -> n p ct s", p=128)

        # the tail-critical final chunks are split into smaller pieces so the
        # last reduce after the last DMA is short; partials fold into extra
        # PSUM-accumulated matmuls
        n_pieces = {(NL - 1, CT - 2): 2, (NL - 1, CT - 1): 8}
        stage_col = 0

        for n in range(NL):
            mm_ops = []
            for ct in range(CT):
                # for the last sample, swap engines so DVE (no
                # read-accumulator step) owns the tail pieces
                use_dve = (ct % 2 == 0) if n < NL - 1 else (ct % 2 == 1)
                pieces = n_pieces.get((n, ct), 1)
                w = S // pieces
                for pi in range(pieces):
                    xt = xpool.tile([128, w], F32, tag="xt")
                    nc.sync.dma_start(out=xt,
                                      in_=xv[n, :, ct, pi * w:(pi + 1) * w])
                    if pieces == 1:
                        dst = xm_sb[:, ct, n:n + 1]
                    else:
                        dst = stage[:, stage_col:stage_col + 1]
                        stage_col += 1
                    mm_ops.append((wgt_sb[:, ct, :], dst))
                    if use_dve:
                        nc.vector.reduce_sum(out=dst, in_=xt,
                                             axis=mybir.AxisListType.X)
                    else:
                        nc.scalar.activation(
                            out=scratch[:, :w], in_=xt, func=ACT_COPY,
                            accum_out=dst)
            # gc[:, n] accumulates on PE as each piece's sum lands
            for i, (lhsT, rhs) in enumerate(mm_ops):
                nc.tensor.matmul(gc_ps[:, n:n + 1], lhsT=lhsT, rhs=rhs,
                                 start=(i == 0), stop=(i == len(mm_ops) - 1))

        # ---- tail: relu -> fc2 halves -> sigmoid -> out DMA (pipelined)
        zr = smalls.tile([J, NL], F32, tag="zr")
        nc.vector.tensor_scalar_max(out=zr, in0=gc_ps, scalar1=0.0)
        res_sb = smalls.tile([NL, C], F32, tag="res")
        half = C // 2
        for h in range(2):  # halves pipeline PE -> ACT -> DMA
            o_ps = psum.tile([NL, half], F32, tag="o")
            nc.tensor.matmul(o_ps, lhsT=zr,
                             rhs=fc2t_sb[:, h * half:(h + 1) * half],
                             start=True, stop=True)
            nc.scalar.activation(out=res_sb[:, h * half:(h + 1) * half],
                                 in_=o_ps, func=ACT_SIGMOID)
            nc.sync.dma_start(out=out_d[:, h * half:(h + 1) * half],
                              in_=res_sb[:, h * half:(h + 1) * half])

    return nc


def _get_nc() -> bass.Bass:
    if "nc" not in _NC_CACHE:
        nc = _build_nc()
        nc.finalize()
        _NC_CACHE["nc"] = nc
    return _NC_CACHE["nc"]


def _prep_inputs(x, e, w1, w2, fc1_w, fc2_w):
    """Host-side shard + weight fold (layout prep only; heavy math on device)."""
    x = np.ascontiguousarray(np.asarray(x, dtype=np.float32)).reshape(N, C, S)

    # fold conv1 / grouped-conv2 / fc1 / (1/S mean) / adjacency-softmax into
    # one [J, C] matrix
    w1d = np.asarray(w1, dtype=np.float64)
    w2g = np.asarray(w2, dtype=np.float64).reshape(G, J, J)
    m2 = np.zeros((CA, CA), dtype=np.float64)
    for g in range(G):
        m2[g * J:(g + 1) * J, g * J:(g + 1) * J] = w2g[g]
    wcomb = np.asarray(fc1_w, np.float64) @ m2 @ (w1d / S)      # [J, C]

    emat = np.full((J * J,), NEG, dtype=np.float64)
    emat[NZ_IDX] = np.asarray(e, dtype=np.float64)[0]
    emat = emat.reshape(J, J)
    emax = emat.max(axis=1, keepdims=True)
    adj = np.exp(emat - emax)
    adj /= adj.sum(axis=1, keepdims=True)

    wg = adj @ wcomb                                            # [J, C]
    wgt = np.ascontiguousarray(
        wg.T.reshape(CT, 128, J).transpose(1, 0, 2)).astype(np.float32)
    fc2t = np.ascontiguousarray(np.asarray(fc2_w, dtype=np.float32).T)

    in_maps = []
    for k in range(NCORES):
        in_maps.append({
            "x": np.ascontiguousarray(x[k * NL:(k + 1) * NL]),
            "wgt": wgt, "fc2t": fc2t,
        })
    return in_maps


def _run(inputs: dict, trace: bool = False, trace_cores=None):
    in_maps = _prep_inputs(**inputs)
    nc = _get_nc()
    res = run_bass_kernel_spmd(nc, in_maps, list(range(NCORES)), trace=trace,
                               trace_cores=trace_cores)
    out = np.concatenate([res.results[k]["out"] for k in range(NCORES)], axis=0)
    return out.reshape(N, C, 1, 1).astype(np.float32), res


def kernel(**inputs) -> np.ndarray:
    out, _ = _run(inputs, trace=False)
    return out


# revision 14
# speedup vs baseline: 1.0166x; 1.0166x over previous
"""Trainium2 Bass kernel for nn_CGCA_branch (gnn_message_passing).

Math: every op between x and the relu is linear and commutes with the global
average pool, so conv1 / grouped-conv2 / fc1 / (1/S mean) *and the adjacency
softmax matmul* all fold on the host into a single [J, C] matrix:

    gc[j, n] = Wg @ sum_s(x[n, :, s]),   Wg = softmax(adj) @ fc1 @ M2 @ (w1/S)

The device kernel is then a 51 MB/core spatial-sum stream (HBM/DMA-bound)
plus tiny per-chunk matmuls, a relu, one [8,512] matmul (split in halves to
pipeline with the sigmoid + store) and a sigmoid.

Streaming: one 1.6 MB DMA per (sample, 128-channel chunk) on the qSync HWDGE
ring, in order; chunk reduces alternate between DVE (reduce_sum) and ACT
(Copy with accum_out) so reduce throughput is ~2.2x one engine and every
engine's in-order queue matches data-arrival order (out-of-order engine
queues were measured to stall the DMA ring on slot-reuse waits).  The last
sample's final chunks are split into small pieces so the post-stream tail is
short.  ACT uses only Copy and Sigmoid - one table set, loaded once at start.

Sharding: pure data parallel - batch 64 split into 8 shards of 8 samples,
one per NeuronCore; weights replicated.
"""

import numpy as np

import concourse.bass as bass
import concourse.bacc as bacc
from concourse import mybir
from concourse.bass_utils import run_bass_kernel_spmd
from concourse.tile import TileContext
from contextlib import ExitStack

# ---- problem constants (hardcoded per harness contract) ----
N, C, H, W = 64, 512, 56, 56
S = H * W                      # 3136 spatial positions
J, CA, G = 17, 272, 16
NCORES = 8
NL = N // NCORES               # 8 samples per core
CT = C // 128                  # 4 channel chunks of 128
NEG = -9e15

_ADJ = np.array([
    [1,1,0,0,0,0,0,0,0,0,0,0,0,0,0,0,0],[1,1,1,0,0,0,0,0,0,0,0,0,0,0,0,0,0],
    [0,1,1,0,0,0,1,0,0,0,0,0,0,0,0,0,0],[0,0,0,1,1,0,1,0,0,0,0,0,0,0,0,0,0],
    [0,0,0,1,1,1,0,0,0,0,0,0,0,0,0,0,0],[0,0,0,0,1,1,0,0,0,0,0,0,0,0,0,0,0],
    [0,0,1,1,0,0,1,1,0,0,0,0,0,0,0,0,0],[0,0,0,0,0,0,1,1,1,0,0,0,0,0,0,0,0],
    [0,0,0,0,0,0,0,1,1,0,0,1,1,0,0,0,1],[0,0,0,0,0,0,0,0,0,1,0,0,0,0,0,0,1],
    [0,0,0,0,0,0,0,0,0,0,1,1,0,0,0,0,0],[0,0,0,0,0,0,0,0,0,0,1,1,1,0,0,0,0],
    [0,0,0,0,0,0,0,0,1,0,0,1,1,0,0,0,0],[0,0,0,0,0,0,0,0,1,0,0,0,0,1,1,0,0],
    [0,0,0,0,0,0,0,0,0,0,0,0,0,1,1,1,0],[0,0,0,0,0,0,0,0,0,0,0,0,0,0,1,1,0],
    [0,0,0,0,0,0,0,0,1,1,0,0,0,0,0,0,1]], dtype=np.int32)
NZ_IDX = np.flatnonzero(_ADJ)  # 49 entries

F32 = mybir.dt.float32
ACT_COPY = mybir.ActivationFunctionType.Copy
ACT_SIGMOID = mybir.ActivationFunctionType.Sigmoid
_NC_CACHE = {}


def _build_nc() -> bass.Bass:
    nc = bacc.Bacc(None, enable_partition_id=False)
    x_d = nc.declare_dram_parameter("x", [NL, C, S], F32, isOutput=False)
    wgt_d = nc.declare_dram_parameter("wgt", [128, CT, J], F32, isOutput=False)
    fc2t_d = nc.declare_dram_parameter("fc2t", [J, C], F32, isOutput=False)
    out_d = nc.declare_dram_parameter("out", [NL, C], F32, isOutput=True)

    with TileContext(nc) as tc, ExitStack() as ctx:
        xpool = ctx.enter_context(tc.tile_pool(name="xpool", bufs=6))
        # sample-7 tiles get fresh slots (no reuse): their DMAs carry no
        # slot-release waits, so the ring never stalls on tail reduces
        cpa = ctx.enter_context(tc.tile_pool(name="cpa", bufs=4))
        cpb = ctx.enter_context(tc.tile_pool(name="cpb", bufs=8))
        singles = ctx.enter_context(tc.tile_pool(name="singles", bufs=1))
        smalls = ctx.enter_context(tc.tile_pool(name="smalls", bufs=3))
        psum = ctx.enter_context(tc.tile_pool(name="psum", bufs=2, space="PSUM"))

        # replicated weights on the SWDGE queue so the HWDGE ring carries
        # only the x stream
        wgt_sb = singles.tile([128, CT, J], F32)
        nc.gpsimd.dma_start(out=wgt_sb, in_=wgt_d[:, :, :])
        fc2t_sb = singles.tile([J, C], F32)
        nc.gpsimd.dma_start(out=fc2t_sb, in_=fc2t_d[:, :])

        # ---- stream x, spatial-sum per (sample, channel-chunk) ----
        xm_sb = singles.tile([128, CT, NL], F32)        # xm[p, ct, n]
        stage = singles.tile([128, 12], F32)            # split-piece partials
        scratch = singles.tile([128, S], F32)           # dummy out for ACT accum
        gc_ps = psum.tile([J, NL], F32, tag="gc")       # gc accumulator
        xv = x_d[:, :, :].rearrange("n (ct p) s -> n p ct s", p=128)

        # the tail-critical final chunks are split into smaller pieces so the
        # last reduce after the last DMA is short; partials fold into extra
        # PSUM-accumulated matmuls
        n_pieces = {(NL - 1, CT - 2): 2, (NL - 1, CT - 1): 8}
        stage_col = 0

        s7i = 0
        for n in range(NL):
            mm_ops = []
            for ct in range(CT):
                # for the last sample, swap engines so DVE (no
                # read-accumulator step) owns the tail pieces
                use_dve = (ct % 2 == 0) if n < NL - 1 else (ct % 2 == 1)
                pieces = n_pieces.get((n, ct), 1)
                w = S // pieces
                for pi in range(pieces):
                    if n < NL - 1:
                        xt = xpool.tile([128, w], F32, tag="xt")
                        nc.sync.dma_start(out=xt,
                                          in_=xv[n, :, ct,
                                                 pi * w:(pi + 1) * w])
                    else:
                        # fresh slot + a late modeled-ready time so the
                        # scheduler keeps these DMAs at the stream tail
                        pool = cpb if pieces > 2 else cpa
                        xt = pool.tile([128, w], F32)
                        with tc.tile_wait_until(1.0 + 0.01 * s7i):
                            nc.sync.dma_start(out=xt,
                                              in_=xv[n, :, ct,
                                                     pi * w:(pi + 1) * w])
                        s7i += 1
                    if pieces == 1:
                        dst = xm_sb[:, ct, n:n + 1]
                    else:
                        dst = stage[:, stage_col:stage_col + 1]
                        stage_col += 1
                    mm_ops.append((wgt_sb[:, ct, :], dst))
                    if use_dve:
                        nc.vector.reduce_sum(out=dst, in_=xt,
                                             axis=mybir.AxisListType.X)
                    else:
                        nc.scalar.activation(
                            out=scratch[:, :w], in_=xt, func=ACT_COPY,
                            accum_out=dst)
            # gc[:, n] accumulates on PE as each piece's sum lands
            for i, (lhsT, rhs) in enumerate(mm_ops):
                nc.tensor.matmul(gc_ps[:, n:n + 1], lhsT=lhsT, rhs=rhs,
                                 start=(i == 0), stop=(i == len(mm_ops) - 1))

        # ---- tail: relu -> fc2 halves -> sigmoid -> out DMA (pipelined)
        zr = smalls.tile([J, NL], F32, tag="zr")
        nc.vector.tensor_scalar_max(out=zr, in0=gc_ps, scalar1=0.0)
        res_sb = smalls.tile([NL, C], F32, tag="res")
        half = C // 2
        for h in range(2):  # halves pipeline PE -> ACT -> DMA
            o_ps = psum.tile([NL, half], F32, tag="o")
            nc.tensor.matmul(o_ps, lhsT=zr,
                             rhs=fc2t_sb[:, h * half:(h + 1) * half],
                             start=True, stop=True)
            nc.scalar.activation(out=res_sb[:, h * half:(h + 1) * half],
                                 in_=o_ps, func=ACT_SIGMOID)
            nc.sync.dma_start(out=out_d[:, h * half:(h + 1) * half],
                              in_=res_sb[:, h * half:(h + 1) * half])

    return nc


def _get_nc() -> bass.Bass:
    if "nc" not in _NC_CACHE:
        nc = _build_nc()
        nc.finalize()
        _NC_CACHE["nc"] = nc
    return _NC_CACHE["nc"]


def _prep_inputs(x, e, w1, w2, fc1_w, fc2_w):
    """Host-side shard + weight fold (layout prep only; heavy math on device)."""
    x = np.ascontiguousarray(np.asarray(x, dtype=np.float32)).reshape(N, C, S)

    # fold conv1 / grouped-conv2 / fc1 / (1/S mean) / adjacency-softmax into
    # one [J, C] matrix
    w1d = np.asarray(w1, dtype=np.float64)
    w2g = np.asarray(w2, dtype=np.float64).reshape(G, J, J)
    m2 = np.zeros((CA, CA), dtype=np.float64)
    for g in range(G):
        m2[g * J:(g + 1) * J, g * J:(g + 1) * J] = w2g[g]
    wcomb = np.asarray(fc1_w, np.float64) @ m2 @ (w1d / S)      # [J, C]

    emat = np.full((J * J,), NEG, dtype=np.float64)
    emat[NZ_IDX] = np.asarray(e, dtype=np.float64)[0]
    emat = emat.reshape(J, J)
    emax = emat.max(axis=1, keepdims=True)
    adj = np.exp(emat - emax)
    adj /= adj.sum(axis=1, keepdims=True)

    wg = adj @ wcomb                                            # [J, C]
    wgt = np.ascontiguousarray(
        wg.T.reshape(CT, 128, J).transpose(1, 0, 2)).astype(np.float32)
    fc2t = np.ascontiguousarray(np.asarray(fc2_w, dtype=np.float32).T)

    in_maps = []
    for k in range(NCORES):
        in_maps.append({
            "x": np.ascontiguousarray(x[k * NL:(k + 1) * NL]),
            "wgt": wgt, "fc2t": fc2t,
        })
    return in_maps


def _run(inputs: dict, trace: bool = False, trace_cores=None):
    in_maps = _prep_inputs(**inputs)
    nc = _get_nc()
    res = run_bass_kernel_spmd(nc, in_maps, list(range(NCORES)), trace=trace,
                               trace_cores=trace_cores)
    out = np.concatenate([res.results[k]["out"] for k in range(NCORES)], axis=0)
    return out.reshape(N, C, 1, 1).astype(np.float32), res


def kernel(**inputs) -> np.ndarray:
    out, _ = _run(inputs, trace=False)
    return out
